# revision 25
# baseline (speedup 1.0000x reference)
"""Trainium2 Bass kernel for nn_Actor_attf (gnn_message_passing).

Data-parallel over batch across 8 NeuronCores; batch chunked into
SCHED pipelined device calls so host encode / upload / execute /
download overlap on the axon link. The end-to-end wall clock is
dominated by the tunnel upload (~55-70 MB/s, LZ-style wire compression
that cannot compress high-entropy bytes), so the wire format is a
6-bit-per-element code: a rational compander
    y = 31*(1+AV)*x / (amax + AV*|x|)        (encode, host numpy)
    x = amax * u / ((1+AV) - AV*|u|)         (decode, device DVE)
whose quantization MSE is within ~4% of the Lloyd-Max optimum for
gaussian inputs, packed 4 codes per 3 bytes as three bit planes
(18.9 MB on the wire vs 96 MB f32). Host encode rounds via the
1.5*2^23 float trick and uses a sampled absmax + clip (clipping rare
tails tightens the steps and slightly improves accuracy). The device
unpacks with int32 SWAR shift/mask ops, decodes with a Newton-iterated
reciprocal bit trick in f32, and scales to bf16. Plane k holds the
lane bytes for subtile-k batch columns, so unpack slices are
contiguous on both host and device.

Output returns as bf16. Consts (weights) live on device across calls,
re-uploaded only if the weight bytes change. The sharded executables
(one per distinct chunk size in SCHED) are traced/compiled once per
process and cached.

Compute pipeline (per 1024-row subtile) is unchanged from the f32r
baseline: block-diag L1/L2 encoders, mask-matmul attention, centered
two-pass LayerNorm with quake-rsqrt, leaky-relu actor head, tanh out.
PE tile-position rules respected: matmul SBUF operands at base
partition {0,32,64} with lhsT.base == rhs.base; psum outputs
quadrant-aligned and zero-padded so no stale PSUM is ever read.
"""
import numpy as np
import ml_dtypes
from contextlib import ExitStack

import concourse.bass as bass
import concourse.tile as tile
from concourse import mybir

F32 = mybir.dt.float32
BF16 = mybir.dt.bfloat16
I8 = mybir.dt.int8
I32 = mybir.dt.int32
AF = mybir.ActivationFunctionType
OP = mybir.AluOpType
BF = ml_dtypes.bfloat16

A, B, OBS, NU, NT = 16, 16384, 96, 32, 16
M = 8              # cores
Bs = B // M        # 2048 batch per core
NSUB = 1024        # rows per subtile (two psum banks)
NMH = 512          # matmul half width (one psum bank)
NMAC = 2048        # rows per macro tile
EPS = 1e-5
QMAGIC = 0x5F3759DF

# Chunk schedule: agents per pipelined device call. Uniform 4-agent
# chunks measured fastest: smaller chunks pay a per-transfer fixed cost
# on the tunnel, larger ones delay the first byte behind the encode.
# Each entry must satisfy gq <= NMAC (ach <= 4) so every macro tile
# decodes a whole number of 6-bit lanes.
SCHED = (4, 4, 4, 4)

# 6-bit wire format: codes q in [1..63] (biased +32), rational compander
#   encode y = 31*(1+AV)*x / (amax + AV*|x|),  q = round(y) + 32
#   decode x = amax * u / ((1+AV) - AV*|u|),   u = (q-32)/31
# Codes packed 4-per-3-bytes as three bit planes; plane k holds the lane
# bytes for subtile-k batch columns, so unpack slices are contiguous on
# both host and device.
AV = 2.5           # compander strength (near Lloyd-optimal for randn)
KREC = 0x7EF127EB  # reciprocal bit-trick magic + 1


def geom(ach):
    """Per-chunk geometry: rows, groups, plane words, macro count."""
    rc = ach * Bs
    gq = rc // 4       # 6-bit groups per partition row
    pw = gq // 4       # int32 words per plane per partition row
    nmac = min(NMAC, rc)
    nmacros = rc // nmac
    return rc, gq, pw, nmac, nmacros


def bd(w, k):
    """block-diag k copies of w."""
    ki, ko = w.shape
    out = np.zeros((ki * k, ko * k), np.float32)
    for g in range(k):
        out[ki * g:ki * (g + 1), ko * g:ko * (g + 1)] = w
    return out


class ColPack:
    """Constant matrices packed as column blocks of one [128, W] array.

    Content placed at rows [row0:row0+k]; kernel slices [sbase:sbase+ssize]."""

    def __init__(self):
        self.cols = []
        self.off = 0
        self.idx = {}

    def add(self, name, arr, row0=0, sbase=0, ssize=None):
        arr = np.asarray(arr, np.float32)
        if arr.ndim == 1:
            arr = arr[:, None]
        k, m = arr.shape
        if ssize is None:
            ssize = row0 + k - sbase
        a = np.zeros((128, m), np.float32)
        a[row0:row0 + k] = arr
        self.idx[name] = (self.off, sbase, ssize, m)
        self.cols.append(a)
        self.off += m

    def pack(self):
        return np.concatenate(self.cols, axis=1)


def build_consts(w):
    """Returns (cpb, cpf): bf16 matmul lhsTs and f32 bias/misc columns.

    L1 lhsTs use natural obs feature order: window A = partitions 0:64
    (self at 0:4, oa pos pairs at 4+2g/5+2g, oa vel pairs at 34+2g/35+2g),
    window B = partitions 64:96 (goal pairs at 64+2g/65+2g)."""
    cpb = ColPack()
    cpf = ColPack()
    oa_w1, oa_w2 = w["oa_w1"], w["oa_w2"]
    g_w1, g_w2 = w["g_w1"], w["g_w2"]
    en_w1, en_w2 = w["en_w1"], w["en_w2"]
    seps = 4.0 * np.sqrt(EPS)

    def l1_oa(groups):
        a = np.zeros((64, 32 * len(groups)), np.float32)
        for j, g in enumerate(groups):
            c = slice(32 * j, 32 * j + 32)
            a[4 + 2 * g, c] = oa_w1[0]
            a[5 + 2 * g, c] = oa_w1[1]
            a[34 + 2 * g, c] = oa_w1[2]
            a[35 + 2 * g, c] = oa_w1[3]
        return a

    def l1_g(groups):
        a = np.zeros((32, 32 * len(groups)), np.float32)
        for j, g in enumerate(groups):
            c = slice(32 * j, 32 * j + 32)
            a[2 * g, c] = g_w1[0]
            a[1 + 2 * g, c] = g_w1[1]
        return a

    l1_self = np.zeros((64, 32), np.float32)
    l1_self[0:4] = en_w1

    # ---- L1 lhsTs ----
    cpb.add("w0a", l1_oa([0, 1, 2, 3]), row0=0, sbase=0, ssize=64)
    cpb.add("w0b", l1_oa([4, 5, 6, 7]), row0=0, sbase=0, ssize=64)
    cpb.add("w1c", l1_oa([8, 9, 10, 11]), row0=0, sbase=0, ssize=64)
    cpb.add("w1d", l1_oa([12, 13, 14]), row0=0, sbase=0, ssize=64)
    cpb.add("w1s", l1_self, row0=0, sbase=0, ssize=64)
    cpb.add("w2a", l1_g([0, 1, 2, 3]), row0=64, sbase=64, ssize=32)
    cpb.add("w2b", l1_g([4, 5, 6, 7]), row0=64, sbase=64, ssize=32)
    cpb.add("w2c", l1_g([8, 9, 10, 11]), row0=64, sbase=64, ssize=32)
    cpb.add("w2d", l1_g([12, 13, 14, 15]), row0=64, sbase=64, ssize=32)
    # ---- L2 lhsTs ----
    cpb.add("lw_oa2", bd(oa_w2, 4))            # [128,64]
    cpb.add("lw_oa2c", bd(oa_w2, 3))           # [96,48]
    cpb.add("lw_en2", en_w2)                   # [32,16]
    cpb.add("lw_g2", bd(g_w2, 4))              # [128,64]
    # ---- attention ----
    r16 = np.zeros((16, 128), np.float32)
    for j in range(8):
        for u in range(16):
            r16[u, 16 * j + u] = 1.0
    cpb.add("r16", r16)
    m8 = np.zeros((128, 32), np.float32)      # scores mask (8 real cols)
    for j in range(8):
        m8[16 * j:16 * j + 16, j] = 1.0
    cpb.add("m8w", m8)
    m8b = np.zeros((112, 32), np.float32)     # oaB: 7 groups at cols 8:15
    for j in range(7):
        m8b[16 * j:16 * j + 16, 8 + j] = 1.0
    cpb.add("m8bw", m8b)
    # e-replicate lhsTs: e lives at psc rows {0:8, 32:40, 64:72, 72:79}
    for nm, base, nj, ncol in [("e_ga", 0, 8, 128), ("e_gb", 32, 8, 128),
                               ("e_oaa", 64, 8, 128), ("e_oab", 72, 7, 112)]:
        e = np.zeros((96, ncol), np.float32)
        for j in range(nj):
            for u in range(16):
                e[base + j, 16 * j + u] = 1.0
        cpb.add(nm, e)
    u16 = np.zeros((128, 32), np.float32)     # centered wsum mask
    for j in range(8):
        for u in range(16):
            for u2 in range(16):
                u16[16 * j + u, u2] = (1.0 if u == u2 else 0.0) - 1.0 / 16.0
    cpb.add("u16w", u16)
    cpb.add("u16bw", u16[:112, :].copy())
    # ---- LN stats (centered two-pass) ----
    stmu = np.zeros((64, 32), np.float32)
    stmu[0:16, 0] = 1.0 / 16.0     # mu_goal
    stmu[32:48, 1] = 1.0 / 16.0    # mu_oa
    cpb.add("stmu", stmu)
    stde = np.zeros((96, 32), np.float32)
    stde[0:8, 0] = seps
    stde[32:40, 0] = seps          # goal denom: e rows 0:8 + 32:40
    stde[64:79, 1] = seps          # oa denom: e rows 64:79
    cpb.add("stdew", stde)
    sts2 = np.zeros((64, 32), np.float32)
    sts2[0:16, 0] = 1.0
    sts2[32:48, 1] = 1.0
    cpb.add("sts2w", sts2)         # sum of (x-mu)^2 -> 16*var
    id2 = np.zeros((2, 32), np.float32)
    id2[0, 0] = 1.0
    id2[1, 1] = 1.0
    cpb.add("id2", id2)            # accumulate De^2 into R
    bcmu = np.zeros((2, 64), np.float32)
    bcmu[0, 0:16] = 1.0
    bcmu[1, 32:48] = 1.0
    cpb.add("bcmu", bcmu)
    bcrg = np.zeros((2, 64), np.float32)
    bcrg[0, 0:16] = 4.0 * w["g_ln_g"]
    bcrg[1, 32:48] = 4.0 * w["oa_ln_g"]
    cpb.add("bcrg", bcrg)          # rstd = 4/sqrt(R16); 4 folded here
    # ---- actor ----
    cpb.add("aw1s", w["a_w1"][0:16])           # [16,32] self part
    aw1a = np.zeros((64, 32), np.float32)
    aw1a[0:16] = w["a_w1"][16:32]              # food
    aw1a[32:48] = w["a_w1"][32:48]             # other
    cpb.add("aw1a", aw1a)
    cpb.add("aw2", w["a_w2"])
    cpb.add("aw3", w["a_w3"])
    # ---- f32 biases + misc ----
    cpf.add("b1_oa", np.tile(w["oa_b1"], 4))
    cpf.add("b1_oac", np.tile(w["oa_b1"], 3))            # [96]
    cpf.add("b1_self", w["en_b1"])                       # [32]
    cpf.add("b1_g", np.tile(w["g_b1"], 4))
    cpf.add("b2_oa", np.tile(w["oa_b2"], 8))
    cpf.add("b2_oab", np.tile(w["oa_b2"], 7))            # [112]
    cpf.add("b2_self", w["en_b2"])                       # [16]
    cpf.add("b2_g", np.tile(w["g_b2"], 8))
    beta64 = np.zeros((64,), np.float32)
    beta64[0:16] = w["g_ln_b"]
    beta64[32:48] = w["oa_ln_b"]
    cpf.add("beta64", beta64)
    cpf.add("ab1", w["a_b1"])
    cpf.add("ab2", w["a_b2"])
    cpf.add("ab3", w["a_b3"])
    cpf.add("qshift", np.full((2, 1), 1, np.int32).view(np.float32))
    cpf.add("qxor", np.full((2, 1), -1, np.int32).view(np.float32))
    cpf.add("qmag", np.full((2, 1), float(QMAGIC + 1), np.float32))
    return cpb, cpf


# ---------------------------------------------------------------- graph
def _emit(nc, tc, ctx, x, xsc, cwb, cwf, out, idxb, idxf, nb, nf, ach,
          dbg=None):
    rc, gq, pw, nmac, nmacros = geom(ach)
    const = ctx.enter_context(tc.tile_pool(name="const", bufs=1))
    ppl = ctx.enter_context(tc.tile_pool(name="ppl", bufs=1))
    pun = ctx.enter_context(tc.tile_pool(name="pun", bufs=1))
    pdec = ctx.enter_context(tc.tile_pool(name="pdec", bufs=1))
    pin = ctx.enter_context(tc.tile_pool(name="pin", bufs=3))
    ph1p = ctx.enter_context(tc.tile_pool(name="ph1", bufs=9))
    penc = ctx.enter_context(tc.tile_pool(name="penc", bufs=6))
    pmul = ctx.enter_context(tc.tile_pool(name="pmul", bufs=6))
    ppn = ctx.enter_context(tc.tile_pool(name="ppn", bufs=6))
    pmid = ctx.enter_context(tc.tile_pool(name="pmid", bufs=2))
    pnar = ctx.enter_context(tc.tile_pool(name="pnar", bufs=2))
    pout = ctx.enter_context(tc.tile_pool(name="pout", bufs=2))
    pps = ctx.enter_context(tc.tile_pool(name="pps", bufs=4, space="PSUM"))

    cwb_s = const.tile([128, nb], BF16)
    nc.gpsimd.dma_start(out=cwb_s, in_=cwb[:, :])
    cwf_s = const.tile([128, nf], F32)
    nc.gpsimd.dma_start(out=cwf_s, in_=cwf[:, :])
    xsc_s = const.tile([128, 1], F32)
    nc.gpsimd.dma_start(out=xsc_s, in_=xsc[:, :])

    def cc(name):
        off, sbase, ssize, m_ = idxb[name]
        return cwb_s[sbase:sbase + ssize, off:off + m_]

    def ccb(name, n):  # f32 bias column, rows 0:n
        off, sbase, ssize, m_ = idxf[name]
        return cwf_s[0:n, off:off + 1]

    def mm(o, lhsT, rhs, start=True, stop=True):
        for h in range(NSUB // NMH):
            nc.tensor.matmul(o[:, h * NMH:(h + 1) * NMH], lhsT,
                             rhs[:, h * NMH:(h + 1) * NMH],
                             start=start, stop=stop)

    def drain_relu(dst, src, bias, n, use_act):
        if use_act:
            nc.scalar.activation(dst, src, AF.Relu, bias=ccb(bias, n))
        else:
            nc.vector.tensor_scalar(out=dst, in0=src, scalar1=ccb(bias, n),
                                    scalar2=0.0, op0=OP.add, op1=OP.max)

    planes = ppl.tile([96, 3 * pw], I32)
    nc.sync.dma_start(out=planes, in_=x[:, :])
    P0w = planes[:, 0:pw]
    P1w = planes[:, pw:2 * pw]
    P2w = planes[:, 2 * pw:3 * pw]

    def unpack_lane(lane):
        """SWAR per-byte 6-bit extraction of lane -> [96, PW] i32."""
        codes = pun.tile([96, pw], I32, tag="codes")
        if lane == 0:
            nc.vector.tensor_scalar(out=codes, in0=P0w, scalar1=0x3F3F3F3F,
                                    scalar2=None, op0=OP.bitwise_and)
        elif lane == 1:
            t1 = pun.tile([96, pw], I32, tag="t1")
            nc.vector.tensor_scalar(out=t1, in0=P0w, scalar1=6,
                                    scalar2=None, op0=OP.logical_shift_right)
            nc.vector.tensor_scalar(out=t1, in0=t1, scalar1=0x03030303,
                                    scalar2=None, op0=OP.bitwise_and)
            t2 = pun.tile([96, pw], I32, tag="t2")
            nc.vector.tensor_scalar(out=t2, in0=P1w, scalar1=0x0F0F0F0F,
                                    scalar2=None, op0=OP.bitwise_and)
            nc.vector.tensor_scalar(out=t2, in0=t2, scalar1=2,
                                    scalar2=None, op0=OP.logical_shift_left)
            nc.vector.tensor_tensor(out=codes, in0=t1, in1=t2,
                                    op=OP.bitwise_or)
        elif lane == 2:
            t1 = pun.tile([96, pw], I32, tag="t1")
            nc.vector.tensor_scalar(out=t1, in0=P1w, scalar1=4,
                                    scalar2=None, op0=OP.logical_shift_right)
            nc.vector.tensor_scalar(out=t1, in0=t1, scalar1=0x0F0F0F0F,
                                    scalar2=None, op0=OP.bitwise_and)
            t2 = pun.tile([96, pw], I32, tag="t2")
            nc.vector.tensor_scalar(out=t2, in0=P2w, scalar1=0x03030303,
                                    scalar2=None, op0=OP.bitwise_and)
            nc.vector.tensor_scalar(out=t2, in0=t2, scalar1=4,
                                    scalar2=None, op0=OP.logical_shift_left)
            nc.vector.tensor_tensor(out=codes, in0=t1, in1=t2,
                                    op=OP.bitwise_or)
        else:
            nc.vector.tensor_scalar(out=codes, in0=P2w, scalar1=2,
                                    scalar2=None, op0=OP.logical_shift_right)
            nc.vector.tensor_scalar(out=codes, in0=codes, scalar1=0x3F3F3F3F,
                                    scalar2=None, op0=OP.bitwise_and)
        return codes

    def decode_lane(lane, xin_half):
        codes = unpack_lane(lane)
        c8 = codes.bitcast(I8)                     # [96, gq]
        uf = pdec.tile([96, gq], F32, tag="uf")
        nc.vector.tensor_scalar(out=uf, in0=c8, scalar1=1.0 / 31.0,
                                scalar2=-32.0 / 31.0, op0=OP.mult, op1=OP.add)
        den = pdec.tile([96, gq], F32, tag="den")
        nc.vector.tensor_scalar(out=den.bitcast(I32), in0=uf.bitcast(I32),
                                scalar1=0x7FFFFFFF, scalar2=None,
                                op0=OP.bitwise_and)
        nc.vector.tensor_scalar(out=den, in0=den, scalar1=-AV,
                                scalar2=1.0 + AV, op0=OP.mult, op1=OP.add)
        r = pdec.tile([96, gq], I32, tag="r")
        nc.vector.tensor_scalar(out=r, in0=den.bitcast(I32), scalar1=-1,
                                scalar2=None, op0=OP.bitwise_xor)
        nc.vector.tensor_scalar(out=r, in0=r, scalar1=KREC,
                                scalar2=None, op0=OP.add)
        rf = r.bitcast(F32)
        tn = pdec.tile([96, gq], F32, tag="tn")
        for _ in range(2):                         # Newton: r <- r*(2-den*r)
            nc.vector.tensor_mul(out=tn, in0=den, in1=rf)
            nc.vector.tensor_scalar(out=tn, in0=tn, scalar1=-1.0,
                                    scalar2=2.0, op0=OP.mult, op1=OP.add)
            nc.vector.tensor_mul(out=rf, in0=rf, in1=tn)
        nc.vector.tensor_mul(out=tn, in0=uf, in1=rf)
        nc.vector.tensor_scalar(out=xin_half, in0=tn,
                                scalar1=xsc_s[0:96, 0:1],
                                scalar2=None, op0=OP.mult)

    for imac in range(nmacros):
        xin = pin.tile([96, nmac], BF16, tag="xin")
        for h in range(nmac // gq):
            decode_lane((nmac // gq) * imac + h,
                        xin[:, h * gq:(h + 1) * gq])
        outw = pout.tile([2, nmac], BF16, tag="outw")

        def dump(name, t, n):
            if dbg is not None and imac == 0 and isub == 0 and name in dbg:
                nc.sync.dma_start(out=dbg[name][:, 0:NSUB], in_=t[0:n, 0:NSUB])

        for isub in range(nmac // NSUB):
            s0 = isub * NSUB
            xs = xin[:, s0:s0 + NSUB]
            xA, xB = xs[0:64, :], xs[64:96, :]

            # ---------------- L1: 9 matmuls, 9 drains ----------------
            h1 = []
            specs = [("w0a", xA, "b1_oa", 128), ("w0b", xA, "b1_oa", 128),
                     ("w1c", xA, "b1_oa", 128), ("w1d", xA, "b1_oac", 96),
                     ("w2a", xB, "b1_g", 128), ("w2b", xB, "b1_g", 128),
                     ("w2c", xB, "b1_g", 128), ("w2d", xB, "b1_g", 128)]
            for i, (lw, xw, bias, npart) in enumerate(specs):
                ps = pps.tile([128, NSUB], F32, tag="ps")
                mm(ps[0:npart, :], cc(lw), xw)
                hs = ph1p.tile([128, NSUB], BF16, tag="h1")
                drain_relu(hs[0:npart, :], ps[0:npart, :], bias, npart,
                           use_act=(i % 2 == 0))
                h1.append(hs)
            psq2 = pps.tile([32, NSUB], F32, tag="ps")
            mm(psq2, cc("w1s"), xA)
            hq = pmid.tile([32, NSUB], BF16, tag="hq")
            drain_relu(hq, psq2, "b1_self", 32, use_act=False)
            dump("h1_0", h1[0], 128)
            dump("hq", hq, 32)

            # ---------------- L2: 9 matmuls, 5 drains ----------------
            psA = pps.tile([128, NSUB], F32, tag="ps")
            mm(psA[0:64, :], cc("lw_oa2"), h1[0])
            mm(psA[64:128, :], cc("lw_oa2"), h1[1])
            encA = penc.tile([128, NSUB], BF16, tag="enc")
            nc.scalar.activation(encA, psA, AF.Relu, bias=ccb("b2_oa", 128))
            psB = pps.tile([128, NSUB], F32, tag="ps")
            mm(psB[0:64, :], cc("lw_oa2"), h1[2])
            mm(psB[64:112, :], cc("lw_oa2c"), h1[3][0:96, :])
            encB = penc.tile([112, NSUB], BF16, tag="encb")
            nc.vector.tensor_scalar(out=encB, in0=psB[0:112, :],
                                    scalar1=ccb("b2_oab", 112), scalar2=0.0,
                                    op0=OP.add, op1=OP.max)
            psq = pps.tile([16, NSUB], F32, tag="ps")
            mm(psq, cc("lw_en2"), hq)
            q_s = pmid.tile([16, NSUB], BF16, tag="qs")
            nc.scalar.activation(q_s, psq, AF.Relu, bias=ccb("b2_self", 16))
            psGA = pps.tile([128, NSUB], F32, tag="ps")
            mm(psGA[0:64, :], cc("lw_g2"), h1[4])
            mm(psGA[64:128, :], cc("lw_g2"), h1[5])
            encGA = penc.tile([128, NSUB], BF16, tag="enc")
            nc.scalar.activation(encGA, psGA, AF.Relu, bias=ccb("b2_g", 128))
            psGB = pps.tile([128, NSUB], F32, tag="ps")
            mm(psGB[0:64, :], cc("lw_g2"), h1[6])
            mm(psGB[64:128, :], cc("lw_g2"), h1[7])
            encGB = penc.tile([128, NSUB], BF16, tag="enc")
            nc.vector.tensor_scalar(out=encGB, in0=psGB,
                                    scalar1=ccb("b2_g", 128), scalar2=0.0,
                                    op0=OP.add, op1=OP.max)
            dump("encA", encA, 128)
            dump("encGA", encGA, 128)
            dump("q_s", q_s, 16)

            # -------------- attention scores -------------------------
            psqr = pps.tile([128, NSUB], F32, tag="ps")
            mm(psqr, cc("r16"), q_s)
            qrep = pmid.tile([128, NSUB], BF16, tag="qrep")
            nc.scalar.activation(qrep, psqr, AF.Copy, scale=0.25)
            psc = pps.tile([96, NSUB], F32, tag="ps")
            pga = pmul.tile([128, NSUB], BF16, tag="pm")
            nc.vector.tensor_mul(out=pga, in0=encGA, in1=qrep)
            mm(psc[0:32, :], cc("m8w"), pga)
            pgb = pmul.tile([128, NSUB], BF16, tag="pm")
            nc.vector.tensor_mul(out=pgb, in0=encGB, in1=qrep)
            mm(psc[32:64, :], cc("m8w"), pgb)
            poa = pmul.tile([128, NSUB], BF16, tag="pm")
            nc.vector.tensor_mul(out=poa, in0=encA, in1=qrep)
            mm(psc[64:96, :], cc("m8w"), poa, start=True, stop=False)
            pob = pmul.tile([112, NSUB], BF16, tag="pm")
            nc.vector.tensor_mul(out=pob, in0=encB, in1=qrep[0:112, :])
            mm(psc[64:96, :], cc("m8bw"), pob, start=False, stop=True)
            e_s = pmid.tile([96, NSUB], BF16, tag="es")
            nc.scalar.activation(e_s, psc, AF.Exp)
            dump("qrep", qrep, 128)
            dump("e_s", e_s, 96)

            # -------------- weighted sums ----------------------------
            att = pps.tile([64, NSUB], F32, tag="ps")
            wspec = [("e_ga", "u16w", encGA, 128, 0, True),
                     ("e_gb", "u16w", encGB, 128, 0, False),
                     ("e_oaa", "u16w", encA, 128, 32, True),
                     ("e_oab", "u16bw", encB, 112, 32, False)]
            for elh, ulh, enc_t, np_, ro, st in wspec:
                per = pps.tile([128, NSUB], F32, tag="ps")
                mm(per[0:np_, :], cc(elh), e_s)
                pp = ppn.tile([128, NSUB], BF16, tag="pp")
                nc.vector.tensor_mul(out=pp[0:np_, :], in0=enc_t,
                                     in1=per[0:np_, :])
                mm(att[ro:ro + 32, :], cc(ulh), pp[0:np_, :],
                   start=st, stop=not st)

            # ---- LN: att is already mean-centered (mask carries -1/16) ----
            d = pmid.tile([64, NSUB], F32, tag="d")
            nc.vector.tensor_scalar_add(out=d, in0=att, scalar1=0.0)
            dump("att", d, 64)
            sqd = pmid.tile([64, NSUB], BF16, tag="sqd")
            nc.scalar.activation(sqd, att, AF.Square)
            psde = pps.tile([32, NSUB], F32, tag="ps")
            mm(psde, cc("stdew"), e_s)
            deb = pnar.tile([2, NSUB], BF16, tag="deb")
            nc.scalar.activation(deb, psde[0:2, :], AF.Copy)
            de2 = pnar.tile([2, NSUB], BF16, tag="de2")
            nc.vector.tensor_mul(out=de2, in0=deb, in1=deb)
            prv = pps.tile([32, NSUB], F32, tag="ps")
            mm(prv, cc("sts2w"), sqd, start=True, stop=False)
            mm(prv, cc("id2"), de2, start=False, stop=True)
            # quake rsqrt + 1 newton step (f32, narrow)
            yi = pnar.tile([2, NSUB], I32, tag="yi")
            nc.vector.tensor_scalar(out=yi, in0=prv[0:2, :].bitcast(I32),
                                    scalar1=ccb("qshift", 2).bitcast(I32),
                                    scalar2=None, op0=OP.logical_shift_right)
            nc.vector.tensor_scalar(out=yi, in0=yi,
                                    scalar1=ccb("qxor", 2).bitcast(I32),
                                    scalar2=None, op0=OP.bitwise_xor)
            nc.vector.tensor_scalar(out=yi, in0=yi, scalar1=ccb("qmag", 2),
                                    scalar2=None, op0=OP.add)
            y0 = yi.bitcast(F32)
            t1 = pnar.tile([2, NSUB], F32, tag="t1")
            nc.vector.tensor_mul(out=t1, in0=y0, in1=y0)
            nc.vector.tensor_mul(out=t1, in0=t1, in1=prv[0:2, :])
            nc.vector.tensor_scalar(out=t1, in0=t1, scalar1=-0.5, scalar2=1.5,
                                    op0=OP.mult, op1=OP.add)
            y1 = pnar.tile([2, NSUB], BF16, tag="y1")
            nc.vector.tensor_mul(out=y1, in0=y0, in1=t1)
            pbr = pps.tile([64, NSUB], F32, tag="ps")
            mm(pbr, cc("bcrg"), y1)
            nc.vector.tensor_mul(out=d, in0=d, in1=pbr)
            mn = pmid.tile([64, NSUB], BF16, tag="mn")
            nc.scalar.activation(mn, d, AF.Relu, bias=ccb("beta64", 64))
            dump("mn", mn, 64)

            # -------------- actor ------------------------------------
            ph = pps.tile([32, NSUB], F32, tag="ps")
            mm(ph, cc("aw1s"), q_s, start=True, stop=False)
            mm(ph, cc("aw1a"), mn, start=False, stop=True)
            a1 = pmid.tile([32, NSUB], BF16, tag="a1")
            nc.scalar.activation(a1, ph, AF.Prelu, bias=ccb("ab1", 32),
                                 alpha=0.01)
            ph2 = pps.tile([32, NSUB], F32, tag="ps")
            mm(ph2, cc("aw2"), a1)
            a2 = pmid.tile([32, NSUB], BF16, tag="a2")
            nc.scalar.activation(a2, ph2, AF.Prelu, bias=ccb("ab2", 32),
                                 alpha=0.01)
            po = pps.tile([2, NSUB], F32, tag="ps")
            mm(po, cc("aw3"), a2)
            nc.scalar.activation(outw[:, s0:s0 + NSUB], po, AF.Tanh,
                                 bias=ccb("ab3", 2))

        nc.sync.dma_start(out=out[:, imac * nmac:(imac + 1) * nmac],
                          in_=outw)


def _split_excess_waits(nc):
    """Walrus in this build can encode only 1 sync wait on Activation/PE
    instruction descriptors. Move extra waits onto a NoOp just before."""
    from concourse import mybir as _mb
    nsplit = 0
    for fn in nc.m.functions:
        for bb in fn.blocks:
            insts = list(bb.instructions)
            out = []
            for ins in insts:
                si = ins.sync_info
                ow = list(si.on_wait) if (si is not None and si.on_wait) else []
                if len(ow) > 1:
                    for w0 in ow[:-1]:
                        nsplit += 1
                        nop = _mb.InstNoOp(
                            name=f"I-wsplit-{nsplit}",
                            engine=ins.engine,
                            ins=[], outs=[],
                            sync_info=_mb.SyncInfo(on_wait=[w0], on_update=[]),
                        )
                        nc.register_instruction(nop, overwrite=True)
                        out.append(nop)
                    ins.sync_info = _mb.SyncInfo(on_wait=[ow[-1]],
                                                 on_update=list(si.on_update or []))
                out.append(ins)
            if len(out) != len(insts):
                bb.instructions = out
    return nsplit


def build(nb, nf, idxb, idxf, ach):
    rc, gq, pw, nmac, nmacros = geom(ach)
    nc = bass.Bass()
    x = nc.dram_tensor("x", [96, 3 * pw], I32, kind="ExternalInput")
    xsc = nc.dram_tensor("xsc", [128, 1], F32, kind="ExternalInput")
    cwb = nc.dram_tensor("cwb", [128, nb], BF16, kind="ExternalInput")
    cwf = nc.dram_tensor("cwf", [128, nf], F32, kind="ExternalInput")
    out = nc.dram_tensor("out", [2, rc], BF16, kind="ExternalOutput")
    with tile.TileContext(nc) as tc, ExitStack() as ctx:
        _emit(nc, tc, ctx, x, xsc, cwb, cwf, out, idxb, idxf, nb, nf, ach)
    _split_excess_waits(nc)
    return nc


# ---------------------------------------------------------------- host API
class _Runtime:
    """Per-process state: bass module, jitted sharded executable, device-
    resident consts, cpu-jitted quantizer."""

    def __init__(self):
        import jax
        import jax.numpy as jnp
        from jax.sharding import Mesh, PartitionSpec, NamedSharding
        from jax.experimental.shard_map import shard_map
        from concourse import bass2jax
        self.jax = jax
        bass2jax.install_neuronx_cc_hook()

        # Bass modules (value-independent; ColPack layout fixed by shapes),
        # one per distinct chunk size in SCHED.
        zeros_w = {k: np.zeros(s, np.float32) for k, s in _WSHAPES.items()}
        cpb, cpf = build_consts(zeros_w)
        self.idxb, self.idxf = cpb.idx, cpf.idx
        self.nb, self.nf = cpb.pack().shape[1], cpf.pack().shape[1]

        devices = jax.devices()[:M]
        self.devices = devices
        self.mesh = Mesh(np.asarray(devices), ("core",))
        self.sh = NamedSharding(self.mesh, PartitionSpec("core"))
        P = PartitionSpec

        def make_run(ach):
            rc = geom(ach)[0]
            nc_ = build(self.nb, self.nf, self.idxb, self.idxf, ach)
            out_avals = [jax.core.ShapedArray((2, rc), np.dtype(BF))]
            pname = (nc_.partition_id_tensor.name
                     if nc_.partition_id_tensor else None)
            in_names = ("x", "xsc", "cwb", "cwf", "out") + (
                (pname,) if pname else ())

            def _body(x, xsc, cwb, cwf, z):
                operands = [x, xsc, cwb, cwf, z]
                if pname is not None:
                    operands.append(bass2jax.partition_id_tensor())
                outs = bass2jax._bass_exec_p.bind(
                    *operands,
                    out_avals=tuple(out_avals),
                    in_names=in_names,
                    out_names=("out",),
                    lowering_input_output_aliases=(),
                    sim_require_finite=True,
                    sim_require_nnan=True,
                    nc=nc_)
                return tuple(outs)

            run = jax.jit(
                shard_map(_body, mesh=self.mesh,
                          in_specs=(P("core"),) * 5, out_specs=(P("core"),),
                          check_rep=False),
                donate_argnums=(4,), keep_unused=True)
            mkzeros = jax.jit(
                lambda: jnp.zeros((M * 2, rc), jnp.bfloat16),
                out_shardings=self.sh)
            return run, mkzeros

        self.runs = {ach: make_run(ach) for ach in sorted(set(SCHED))}
        self.wkey = None
        self.cwb_d = None
        self.cwf_d = None

        # numpy quantize scratch per chunk size; per-(chunk,core) block
        # stays L2-resident
        self._scr = {}
        for ach in set(SCHED):
            rc, gq, pw, _, _ = geom(ach)
            self._scr[ach] = (
                np.empty((ach, Bs, OBS), np.float32),    # den
                np.empty((ach, Bs, OBS), np.float32),    # num
                np.empty((ach * Bs, OBS), np.uint8),     # tmp8
                np.empty((OBS, rc), np.uint8),           # ct
                np.empty((OBS, gq), np.uint8),           # t1
                np.empty((OBS, gq), np.uint8),           # t2
            )

    def prep_np(self, s, a0, ach):
        """Encode agents [a0, a0+ach) of s [A,B,OBS] f32 -> (xp
        [M*96, 3*pw] int32 bit-planes of 6-bit companded codes,
        xsc [M*128,1] f32 = per-core scale).

        Scale is a sampled absmax (stride 64); the explicit clip bounds
        |y| <= 31 so the 1.5*2^23 round trick stays exact. Clipping the
        few unsampled outliers slightly tightens the quant steps and
        measurably improves accuracy vs the exact absmax."""
        rc, gq, pw, _, _ = geom(ach)
        den, num, tmp8, ct, t1, t2 = self._scr[ach]
        xp = np.empty((M * OBS, 3 * pw), np.int32)
        xpu = xp.view(np.uint8).reshape(M * OBS, 3 * gq)
        xsc = np.empty((M * 128, 1), np.float32)
        iv = num.view(np.int32)
        t8 = tmp8.reshape(ach, Bs, OBS)
        for m in range(M):
            src = s[a0:a0 + ach, m * Bs:(m + 1) * Bs, :]
            sub = src.reshape(-1)[::64]
            amax = max(float(sub.max()), -float(sub.min()), 1e-20)
            # y = c1*x/(d + |x|) with d = amax/AV, c1 = 31*(1+AV)/AV
            np.abs(src, out=den)
            np.add(den, np.float32(amax / AV), out=den)
            np.multiply(src, np.float32(31.0 * (1.0 + AV) / AV), out=num)
            np.divide(num, den, out=num)
            np.clip(num, -31.0, 31.0, out=num)
            np.add(num, np.float32(12582912.0), out=num)
            np.subtract(iv, np.int32(0x4B400000 - 32), out=iv)
            np.copyto(t8, iv, casting='unsafe')
            ct[...] = tmp8.T
            A0 = ct[:, 0:gq]
            A1 = ct[:, gq:2 * gq]
            A2 = ct[:, 2 * gq:3 * gq]
            A3 = ct[:, 3 * gq:4 * gq]
            W = xpu[m * OBS:(m + 1) * OBS]
            np.left_shift(A1, 6, out=t1)
            np.bitwise_or(A0, t1, out=W[:, 0:gq])
            np.right_shift(A1, 2, out=t1)
            np.left_shift(A2, 4, out=t2)
            np.bitwise_or(t1, t2, out=W[:, gq:2 * gq])
            np.right_shift(A2, 4, out=t1)
            np.left_shift(A3, 2, out=t2)
            np.bitwise_or(t1, t2, out=W[:, 2 * gq:3 * gq])
            xsc[m * 128:(m + 1) * 128] = np.float32(amax)
        return xp, xsc

    def prep_put_early(self, s, a0, ach):
        """Like prep_np, but device_puts each core's block to its device
        as soon as it is packed, so the first bytes hit the axon link
        ~3 ms in instead of after the whole chunk is encoded. Returns
        committed sharded jax arrays assembled zero-copy from the
        per-device pieces."""
        jax = self.jax
        rc, gq, pw, _, _ = geom(ach)
        xp, xsc = None, None
        den, num, tmp8, ct, t1, t2 = self._scr[ach]
        iv = num.view(np.int32)
        t8 = tmp8.reshape(ach, Bs, OBS)
        blk = np.empty((OBS, 3 * pw), np.int32)
        blku = blk.view(np.uint8).reshape(OBS, 3 * gq)
        scb = np.empty((128, 1), np.float32)
        ds, dscs = [], []
        for m in range(M):
            src = s[a0:a0 + ach, m * Bs:(m + 1) * Bs, :]
            sub = src.reshape(-1)[::64]
            amax = max(float(sub.max()), -float(sub.min()), 1e-20)
            np.abs(src, out=den)
            np.add(den, np.float32(amax / AV), out=den)
            np.multiply(src, np.float32(31.0 * (1.0 + AV) / AV), out=num)
            np.divide(num, den, out=num)
            np.clip(num, -31.0, 31.0, out=num)
            np.add(num, np.float32(12582912.0), out=num)
            np.subtract(iv, np.int32(0x4B400000 - 32), out=iv)
            np.copyto(t8, iv, casting='unsafe')
            ct[...] = tmp8.T
            A0 = ct[:, 0:gq]
            A1 = ct[:, gq:2 * gq]
            A2 = ct[:, 2 * gq:3 * gq]
            A3 = ct[:, 3 * gq:4 * gq]
            np.left_shift(A1, 6, out=t1)
            np.bitwise_or(A0, t1, out=blku[:, 0:gq])
            np.right_shift(A1, 2, out=t1)
            np.left_shift(A2, 4, out=t2)
            np.bitwise_or(t1, t2, out=blku[:, gq:2 * gq])
            np.right_shift(A2, 4, out=t1)
            np.left_shift(A3, 2, out=t2)
            np.bitwise_or(t1, t2, out=blku[:, 2 * gq:3 * gq])
            scb[:] = np.float32(amax)
            ds.append(jax.device_put(blk, self.devices[m]))
            dscs.append(jax.device_put(scb, self.devices[m]))
        xp = jax.make_array_from_single_device_arrays(
            (M * OBS, 3 * pw), self.sh, ds)
        xsc = jax.make_array_from_single_device_arrays(
            (M * 128, 1), self.sh, dscs)
        return xp, xsc

    def set_weights(self, w):
        import hashlib
        h = hashlib.blake2b(digest_size=16)
        for k in sorted(_WSHAPES):
            h.update(np.ascontiguousarray(w[k], np.float32).tobytes())
        key = h.digest()
        if key == self.wkey:
            return
        cpb, cpf = build_consts(w)
        assert cpb.idx == self.idxb and cpf.idx == self.idxf
        cwb = np.ascontiguousarray(cpb.pack().astype(BF))
        cwf = np.ascontiguousarray(cpf.pack())
        self.cwb_d = self.jax.device_put(
            np.concatenate([cwb] * M, axis=0), self.sh)
        self.cwf_d = self.jax.device_put(
            np.concatenate([cwf] * M, axis=0), self.sh)
        self.jax.block_until_ready((self.cwb_d, self.cwf_d))
        self.wkey = key


_WSHAPES = {
    "en_w1": (4, NU), "en_b1": (NU,), "en_w2": (NU, NT), "en_b2": (NT,),
    "oa_w1": (4, NU), "oa_b1": (NU,), "oa_w2": (NU, NT), "oa_b2": (NT,),
    "oa_ln_g": (NT,), "oa_ln_b": (NT,),
    "g_w1": (2, NU), "g_b1": (NU,), "g_w2": (NU, NT), "g_b2": (NT,),
    "g_ln_g": (NT,), "g_ln_b": (NT,),
    "a_w1": (NT * 3, NU), "a_b1": (NU,), "a_w2": (NU, NU), "a_b2": (NU,),
    "a_w3": (NU, 2), "a_b3": (2,),
}

_rt = None


def _runtime():
    global _rt
    if _rt is None:
        _rt = _Runtime()
    return _rt


def kernel(**inputs):
    rt = _runtime()
    w = {k: np.asarray(inputs[k], np.float32) for k in _WSHAPES}
    rt.set_weights(w)
    s = np.asarray(inputs["s_input"], np.float32)

    # Interleaved pipeline: encode chunk c (numpy, mostly overlapping the
    # axon link streaming earlier chunks), enqueue its device call
    # immediately so the network never starves. The jit enqueue copies xp
    # into PJRT staging synchronously, so the numpy scratch buffers are
    # free to reuse right after run() returns.
    # Pre-dispatch the device-side zero-fills for every chunk: they cost
    # no wire bytes and execute remotely while the host encodes chunk 0.
    zs = [rt.runs[ach][1]() for ach in SCHED]

    outs = []
    a0 = 0
    for ci, ach in enumerate(SCHED):
        if ci == 0:
            xp, xsc = rt.prep_put_early(s, a0, ach)
        else:
            xp, xsc = rt.prep_np(s, a0, ach)
        run, _ = rt.runs[ach]
        (o,) = run(xp, xsc, rt.cwb_d, rt.cwf_d, zs[ci])
        o.copy_to_host_async()
        outs.append((a0, ach, o))
        a0 += ach

    res = np.empty((A, B, 2), np.float32)
    for a0, ach, o in outs:
        v = np.asarray(o).astype(np.float32)          # [M*2, rc]
        v = v.reshape(M, 2, ach, Bs)                  # [m, j, a_l, b]
        res[a0:a0 + ach] = v.transpose(2, 0, 3, 1).reshape(ach, B, 2)
    return res



# revision 26
# speedup vs baseline: 1.1220x; 1.1220x over previous
"""Trainium2 Bass kernel for nn_Actor_attf (gnn_message_passing).

Data-parallel over batch across 8 NeuronCores; batch chunked into
SCHED pipelined device calls so host encode / upload / execute /
download overlap on the axon link. The end-to-end wall clock is
dominated by the tunnel upload (~55-70 MB/s, LZ-style wire compression
that cannot compress high-entropy bytes), so the wire format is a
6-bit-per-element code: a rational compander
    y = 31*(1+AV)*x / (amax + AV*|x|)        (encode, host numpy)
    x = amax * u / ((1+AV) - AV*|u|)         (decode, device DVE)
whose quantization MSE is within ~4% of the Lloyd-Max optimum for
gaussian inputs, packed 4 codes per 3 bytes as three bit planes
(18.9 MB on the wire vs 96 MB f32). Host encode rounds via the
1.5*2^23 float trick and uses a sampled absmax + clip (clipping rare
tails tightens the steps and slightly improves accuracy). The device
unpacks with int32 SWAR shift/mask ops, decodes with a Newton-iterated
reciprocal bit trick in f32, and scales to bf16. Plane k holds the
lane bytes for subtile-k batch columns, so unpack slices are
contiguous on both host and device.

Output returns as bf16. Consts (weights) live on device across calls,
re-uploaded only if the weight bytes change. The sharded executables
(one per distinct chunk size in SCHED) are traced/compiled once per
process and cached.

Compute pipeline (per 1024-row subtile) is unchanged from the f32r
baseline: block-diag L1/L2 encoders, mask-matmul attention, centered
two-pass LayerNorm with quake-rsqrt, leaky-relu actor head, tanh out.
PE tile-position rules respected: matmul SBUF operands at base
partition {0,32,64} with lhsT.base == rhs.base; psum outputs
quadrant-aligned and zero-padded so no stale PSUM is ever read.
"""
import numpy as np
import ml_dtypes
from contextlib import ExitStack

import concourse.bass as bass
import concourse.tile as tile
from concourse import mybir

F32 = mybir.dt.float32
BF16 = mybir.dt.bfloat16
I8 = mybir.dt.int8
I32 = mybir.dt.int32
AF = mybir.ActivationFunctionType
OP = mybir.AluOpType
BF = ml_dtypes.bfloat16

A, B, OBS, NU, NT = 16, 16384, 96, 32, 16
M = 8              # cores
Bs = B // M        # 2048 batch per core
NSUB = 1024        # rows per subtile (two psum banks)
NMH = 512          # matmul half width (one psum bank)
NMAC = 2048        # rows per macro tile
EPS = 1e-5
QMAGIC = 0x5F3759DF

# Chunk schedule: agents per pipelined device call. Uniform 4-agent
# chunks measured fastest: smaller chunks pay a per-transfer fixed cost
# on the tunnel, larger ones delay the first byte behind the encode.
# Each entry must satisfy gq <= NMAC (ach <= 4) so every macro tile
# decodes a whole number of 6-bit lanes.
SCHED = (4, 4, 4, 4)

# 6-bit wire format: codes q in [1..63] (biased +32), rational compander
#   encode y = 31*(1+AV)*x / (amax + AV*|x|),  q = round(y) + 32
#   decode x = amax * u / ((1+AV) - AV*|u|),   u = (q-32)/31
# Codes packed 4-per-3-bytes as three bit planes; plane k holds the lane
# bytes for subtile-k batch columns, so unpack slices are contiguous on
# both host and device.
AV = 2.5           # compander strength (near Lloyd-optimal for randn)
KREC = 0x7EF127EB  # reciprocal bit-trick magic + 1


def geom(ach):
    """Per-chunk geometry: rows, groups, plane words, macro count."""
    rc = ach * Bs
    gq = rc // 4       # 6-bit groups per partition row
    pw = gq // 4       # int32 words per plane per partition row
    nmac = min(NMAC, rc)
    nmacros = rc // nmac
    return rc, gq, pw, nmac, nmacros


def bd(w, k):
    """block-diag k copies of w."""
    ki, ko = w.shape
    out = np.zeros((ki * k, ko * k), np.float32)
    for g in range(k):
        out[ki * g:ki * (g + 1), ko * g:ko * (g + 1)] = w
    return out


class ColPack:
    """Constant matrices packed as column blocks of one [128, W] array.

    Content placed at rows [row0:row0+k]; kernel slices [sbase:sbase+ssize]."""

    def __init__(self):
        self.cols = []
        self.off = 0
        self.idx = {}

    def add(self, name, arr, row0=0, sbase=0, ssize=None):
        arr = np.asarray(arr, np.float32)
        if arr.ndim == 1:
            arr = arr[:, None]
        k, m = arr.shape
        if ssize is None:
            ssize = row0 + k - sbase
        a = np.zeros((128, m), np.float32)
        a[row0:row0 + k] = arr
        self.idx[name] = (self.off, sbase, ssize, m)
        self.cols.append(a)
        self.off += m

    def pack(self):
        return np.concatenate(self.cols, axis=1)


def build_consts(w):
    """Returns (cpb, cpf): bf16 matmul lhsTs and f32 bias/misc columns.

    L1 lhsTs use natural obs feature order: window A = partitions 0:64
    (self at 0:4, oa pos pairs at 4+2g/5+2g, oa vel pairs at 34+2g/35+2g),
    window B = partitions 64:96 (goal pairs at 64+2g/65+2g)."""
    cpb = ColPack()
    cpf = ColPack()
    oa_w1, oa_w2 = w["oa_w1"], w["oa_w2"]
    g_w1, g_w2 = w["g_w1"], w["g_w2"]
    en_w1, en_w2 = w["en_w1"], w["en_w2"]
    seps = 4.0 * np.sqrt(EPS)

    def l1_oa(groups):
        a = np.zeros((64, 32 * len(groups)), np.float32)
        for j, g in enumerate(groups):
            c = slice(32 * j, 32 * j + 32)
            a[4 + 2 * g, c] = oa_w1[0]
            a[5 + 2 * g, c] = oa_w1[1]
            a[34 + 2 * g, c] = oa_w1[2]
            a[35 + 2 * g, c] = oa_w1[3]
        return a

    def l1_g(groups):
        a = np.zeros((32, 32 * len(groups)), np.float32)
        for j, g in enumerate(groups):
            c = slice(32 * j, 32 * j + 32)
            a[2 * g, c] = g_w1[0]
            a[1 + 2 * g, c] = g_w1[1]
        return a

    l1_self = np.zeros((64, 32), np.float32)
    l1_self[0:4] = en_w1

    # ---- L1 lhsTs ----
    cpb.add("w0a", l1_oa([0, 1, 2, 3]), row0=0, sbase=0, ssize=64)
    cpb.add("w0b", l1_oa([4, 5, 6, 7]), row0=0, sbase=0, ssize=64)
    cpb.add("w1c", l1_oa([8, 9, 10, 11]), row0=0, sbase=0, ssize=64)
    cpb.add("w1d", l1_oa([12, 13, 14]), row0=0, sbase=0, ssize=64)
    cpb.add("w1s", l1_self, row0=0, sbase=0, ssize=64)
    cpb.add("w2a", l1_g([0, 1, 2, 3]), row0=64, sbase=64, ssize=32)
    cpb.add("w2b", l1_g([4, 5, 6, 7]), row0=64, sbase=64, ssize=32)
    cpb.add("w2c", l1_g([8, 9, 10, 11]), row0=64, sbase=64, ssize=32)
    cpb.add("w2d", l1_g([12, 13, 14, 15]), row0=64, sbase=64, ssize=32)
    # ---- L2 lhsTs ----
    cpb.add("lw_oa2", bd(oa_w2, 4))            # [128,64]
    cpb.add("lw_oa2c", bd(oa_w2, 3))           # [96,48]
    cpb.add("lw_en2", en_w2)                   # [32,16]
    cpb.add("lw_g2", bd(g_w2, 4))              # [128,64]
    # ---- attention ----
    r16 = np.zeros((16, 128), np.float32)
    for j in range(8):
        for u in range(16):
            r16[u, 16 * j + u] = 1.0
    cpb.add("r16", r16)
    m8 = np.zeros((128, 32), np.float32)      # scores mask (8 real cols)
    for j in range(8):
        m8[16 * j:16 * j + 16, j] = 1.0
    cpb.add("m8w", m8)
    m8b = np.zeros((112, 32), np.float32)     # oaB: 7 groups at cols 8:15
    for j in range(7):
        m8b[16 * j:16 * j + 16, 8 + j] = 1.0
    cpb.add("m8bw", m8b)
    # e-replicate lhsTs: e lives at psc rows {0:8, 32:40, 64:72, 72:79}
    for nm, base, nj, ncol in [("e_ga", 0, 8, 128), ("e_gb", 32, 8, 128),
                               ("e_oaa", 64, 8, 128), ("e_oab", 72, 7, 112)]:
        e = np.zeros((96, ncol), np.float32)
        for j in range(nj):
            for u in range(16):
                e[base + j, 16 * j + u] = 1.0
        cpb.add(nm, e)
    u16 = np.zeros((128, 32), np.float32)     # centered wsum mask
    for j in range(8):
        for u in range(16):
            for u2 in range(16):
                u16[16 * j + u, u2] = (1.0 if u == u2 else 0.0) - 1.0 / 16.0
    cpb.add("u16w", u16)
    cpb.add("u16bw", u16[:112, :].copy())
    # ---- LN stats (centered two-pass) ----
    stmu = np.zeros((64, 32), np.float32)
    stmu[0:16, 0] = 1.0 / 16.0     # mu_goal
    stmu[32:48, 1] = 1.0 / 16.0    # mu_oa
    cpb.add("stmu", stmu)
    stde = np.zeros((96, 32), np.float32)
    stde[0:8, 0] = seps
    stde[32:40, 0] = seps          # goal denom: e rows 0:8 + 32:40
    stde[64:79, 1] = seps          # oa denom: e rows 64:79
    cpb.add("stdew", stde)
    sts2 = np.zeros((64, 32), np.float32)
    sts2[0:16, 0] = 1.0
    sts2[32:48, 1] = 1.0
    cpb.add("sts2w", sts2)         # sum of (x-mu)^2 -> 16*var
    id2 = np.zeros((2, 32), np.float32)
    id2[0, 0] = 1.0
    id2[1, 1] = 1.0
    cpb.add("id2", id2)            # accumulate De^2 into R
    bcmu = np.zeros((2, 64), np.float32)
    bcmu[0, 0:16] = 1.0
    bcmu[1, 32:48] = 1.0
    cpb.add("bcmu", bcmu)
    bcrg = np.zeros((2, 64), np.float32)
    bcrg[0, 0:16] = 4.0 * w["g_ln_g"]
    bcrg[1, 32:48] = 4.0 * w["oa_ln_g"]
    cpb.add("bcrg", bcrg)          # rstd = 4/sqrt(R16); 4 folded here
    # ---- actor ----
    cpb.add("aw1s", w["a_w1"][0:16])           # [16,32] self part
    aw1a = np.zeros((64, 32), np.float32)
    aw1a[0:16] = w["a_w1"][16:32]              # food
    aw1a[32:48] = w["a_w1"][32:48]             # other
    cpb.add("aw1a", aw1a)
    cpb.add("aw2", w["a_w2"])
    cpb.add("aw3", w["a_w3"])
    # ---- f32 biases + misc ----
    cpf.add("b1_oa", np.tile(w["oa_b1"], 4))
    cpf.add("b1_oac", np.tile(w["oa_b1"], 3))            # [96]
    cpf.add("b1_self", w["en_b1"])                       # [32]
    cpf.add("b1_g", np.tile(w["g_b1"], 4))
    cpf.add("b2_oa", np.tile(w["oa_b2"], 8))
    cpf.add("b2_oab", np.tile(w["oa_b2"], 7))            # [112]
    cpf.add("b2_self", w["en_b2"])                       # [16]
    cpf.add("b2_g", np.tile(w["g_b2"], 8))
    beta64 = np.zeros((64,), np.float32)
    beta64[0:16] = w["g_ln_b"]
    beta64[32:48] = w["oa_ln_b"]
    cpf.add("beta64", beta64)
    cpf.add("ab1", w["a_b1"])
    cpf.add("ab2", w["a_b2"])
    cpf.add("ab3", w["a_b3"])
    cpf.add("qshift", np.full((2, 1), 1, np.int32).view(np.float32))
    cpf.add("qxor", np.full((2, 1), -1, np.int32).view(np.float32))
    cpf.add("qmag", np.full((2, 1), float(QMAGIC + 1), np.float32))
    return cpb, cpf


# ---------------------------------------------------------------- graph
def _emit(nc, tc, ctx, x, xsc, cwb, cwf, out, idxb, idxf, nb, nf, ach,
          dbg=None):
    rc, gq, pw, nmac, nmacros = geom(ach)
    const = ctx.enter_context(tc.tile_pool(name="const", bufs=1))
    ppl = ctx.enter_context(tc.tile_pool(name="ppl", bufs=1))
    pun = ctx.enter_context(tc.tile_pool(name="pun", bufs=1))
    pdec = ctx.enter_context(tc.tile_pool(name="pdec", bufs=1))
    pin = ctx.enter_context(tc.tile_pool(name="pin", bufs=3))
    ph1p = ctx.enter_context(tc.tile_pool(name="ph1", bufs=9))
    penc = ctx.enter_context(tc.tile_pool(name="penc", bufs=6))
    pmul = ctx.enter_context(tc.tile_pool(name="pmul", bufs=6))
    ppn = ctx.enter_context(tc.tile_pool(name="ppn", bufs=6))
    pmid = ctx.enter_context(tc.tile_pool(name="pmid", bufs=2))
    pnar = ctx.enter_context(tc.tile_pool(name="pnar", bufs=2))
    pout = ctx.enter_context(tc.tile_pool(name="pout", bufs=2))
    pps = ctx.enter_context(tc.tile_pool(name="pps", bufs=4, space="PSUM"))

    cwb_s = const.tile([128, nb], BF16)
    nc.gpsimd.dma_start(out=cwb_s, in_=cwb[:, :])
    cwf_s = const.tile([128, nf], F32)
    nc.gpsimd.dma_start(out=cwf_s, in_=cwf[:, :])
    xsc_s = const.tile([128, 1], F32)
    nc.gpsimd.dma_start(out=xsc_s, in_=xsc[:, :])

    def cc(name):
        off, sbase, ssize, m_ = idxb[name]
        return cwb_s[sbase:sbase + ssize, off:off + m_]

    def ccb(name, n):  # f32 bias column, rows 0:n
        off, sbase, ssize, m_ = idxf[name]
        return cwf_s[0:n, off:off + 1]

    def mm(o, lhsT, rhs, start=True, stop=True):
        for h in range(NSUB // NMH):
            nc.tensor.matmul(o[:, h * NMH:(h + 1) * NMH], lhsT,
                             rhs[:, h * NMH:(h + 1) * NMH],
                             start=start, stop=stop)

    def drain_relu(dst, src, bias, n, use_act):
        if use_act:
            nc.scalar.activation(dst, src, AF.Relu, bias=ccb(bias, n))
        else:
            nc.vector.tensor_scalar(out=dst, in0=src, scalar1=ccb(bias, n),
                                    scalar2=0.0, op0=OP.add, op1=OP.max)

    planes = ppl.tile([96, 3 * pw], I32)
    nc.sync.dma_start(out=planes, in_=x[:, :])
    P0w = planes[:, 0:pw]
    P1w = planes[:, pw:2 * pw]
    P2w = planes[:, 2 * pw:3 * pw]

    def unpack_lane(lane):
        """SWAR per-byte 6-bit extraction of lane -> [96, PW] i32."""
        codes = pun.tile([96, pw], I32, tag="codes")
        if lane == 0:
            nc.vector.tensor_scalar(out=codes, in0=P0w, scalar1=0x3F3F3F3F,
                                    scalar2=None, op0=OP.bitwise_and)
        elif lane == 1:
            t1 = pun.tile([96, pw], I32, tag="t1")
            nc.vector.tensor_scalar(out=t1, in0=P0w, scalar1=6,
                                    scalar2=None, op0=OP.logical_shift_right)
            nc.vector.tensor_scalar(out=t1, in0=t1, scalar1=0x03030303,
                                    scalar2=None, op0=OP.bitwise_and)
            t2 = pun.tile([96, pw], I32, tag="t2")
            nc.vector.tensor_scalar(out=t2, in0=P1w, scalar1=0x0F0F0F0F,
                                    scalar2=None, op0=OP.bitwise_and)
            nc.vector.tensor_scalar(out=t2, in0=t2, scalar1=2,
                                    scalar2=None, op0=OP.logical_shift_left)
            nc.vector.tensor_tensor(out=codes, in0=t1, in1=t2,
                                    op=OP.bitwise_or)
        elif lane == 2:
            t1 = pun.tile([96, pw], I32, tag="t1")
            nc.vector.tensor_scalar(out=t1, in0=P1w, scalar1=4,
                                    scalar2=None, op0=OP.logical_shift_right)
            nc.vector.tensor_scalar(out=t1, in0=t1, scalar1=0x0F0F0F0F,
                                    scalar2=None, op0=OP.bitwise_and)
            t2 = pun.tile([96, pw], I32, tag="t2")
            nc.vector.tensor_scalar(out=t2, in0=P2w, scalar1=0x03030303,
                                    scalar2=None, op0=OP.bitwise_and)
            nc.vector.tensor_scalar(out=t2, in0=t2, scalar1=4,
                                    scalar2=None, op0=OP.logical_shift_left)
            nc.vector.tensor_tensor(out=codes, in0=t1, in1=t2,
                                    op=OP.bitwise_or)
        else:
            nc.vector.tensor_scalar(out=codes, in0=P2w, scalar1=2,
                                    scalar2=None, op0=OP.logical_shift_right)
            nc.vector.tensor_scalar(out=codes, in0=codes, scalar1=0x3F3F3F3F,
                                    scalar2=None, op0=OP.bitwise_and)
        return codes

    def decode_lane(lane, xin_half):
        codes = unpack_lane(lane)
        c8 = codes.bitcast(I8)                     # [96, gq]
        uf = pdec.tile([96, gq], F32, tag="uf")
        nc.vector.tensor_scalar(out=uf, in0=c8, scalar1=1.0 / 31.0,
                                scalar2=-32.0 / 31.0, op0=OP.mult, op1=OP.add)
        den = pdec.tile([96, gq], F32, tag="den")
        nc.vector.tensor_scalar(out=den.bitcast(I32), in0=uf.bitcast(I32),
                                scalar1=0x7FFFFFFF, scalar2=None,
                                op0=OP.bitwise_and)
        nc.vector.tensor_scalar(out=den, in0=den, scalar1=-AV,
                                scalar2=1.0 + AV, op0=OP.mult, op1=OP.add)
        r = pdec.tile([96, gq], I32, tag="r")
        nc.vector.tensor_scalar(out=r, in0=den.bitcast(I32), scalar1=-1,
                                scalar2=None, op0=OP.bitwise_xor)
        nc.vector.tensor_scalar(out=r, in0=r, scalar1=KREC,
                                scalar2=None, op0=OP.add)
        rf = r.bitcast(F32)
        tn = pdec.tile([96, gq], F32, tag="tn")
        for _ in range(2):                         # Newton: r <- r*(2-den*r)
            nc.vector.tensor_mul(out=tn, in0=den, in1=rf)
            nc.vector.tensor_scalar(out=tn, in0=tn, scalar1=-1.0,
                                    scalar2=2.0, op0=OP.mult, op1=OP.add)
            nc.vector.tensor_mul(out=rf, in0=rf, in1=tn)
        nc.vector.tensor_mul(out=tn, in0=uf, in1=rf)
        nc.vector.tensor_scalar(out=xin_half, in0=tn,
                                scalar1=xsc_s[0:96, 0:1],
                                scalar2=None, op0=OP.mult)

    for imac in range(nmacros):
        xin = pin.tile([96, nmac], BF16, tag="xin")
        for h in range(nmac // gq):
            decode_lane((nmac // gq) * imac + h,
                        xin[:, h * gq:(h + 1) * gq])
        outw = pout.tile([2, nmac], BF16, tag="outw")

        def dump(name, t, n):
            if dbg is not None and imac == 0 and isub == 0 and name in dbg:
                nc.sync.dma_start(out=dbg[name][:, 0:NSUB], in_=t[0:n, 0:NSUB])

        for isub in range(nmac // NSUB):
            s0 = isub * NSUB
            xs = xin[:, s0:s0 + NSUB]
            xA, xB = xs[0:64, :], xs[64:96, :]

            # ---------------- L1: 9 matmuls, 9 drains ----------------
            h1 = []
            specs = [("w0a", xA, "b1_oa", 128), ("w0b", xA, "b1_oa", 128),
                     ("w1c", xA, "b1_oa", 128), ("w1d", xA, "b1_oac", 96),
                     ("w2a", xB, "b1_g", 128), ("w2b", xB, "b1_g", 128),
                     ("w2c", xB, "b1_g", 128), ("w2d", xB, "b1_g", 128)]
            for i, (lw, xw, bias, npart) in enumerate(specs):
                ps = pps.tile([128, NSUB], F32, tag="ps")
                mm(ps[0:npart, :], cc(lw), xw)
                hs = ph1p.tile([128, NSUB], BF16, tag="h1")
                drain_relu(hs[0:npart, :], ps[0:npart, :], bias, npart,
                           use_act=(i % 2 == 0))
                h1.append(hs)
            psq2 = pps.tile([32, NSUB], F32, tag="ps")
            mm(psq2, cc("w1s"), xA)
            hq = pmid.tile([32, NSUB], BF16, tag="hq")
            drain_relu(hq, psq2, "b1_self", 32, use_act=False)
            dump("h1_0", h1[0], 128)
            dump("hq", hq, 32)

            # ---------------- L2: 9 matmuls, 5 drains ----------------
            psA = pps.tile([128, NSUB], F32, tag="ps")
            mm(psA[0:64, :], cc("lw_oa2"), h1[0])
            mm(psA[64:128, :], cc("lw_oa2"), h1[1])
            encA = penc.tile([128, NSUB], BF16, tag="enc")
            nc.scalar.activation(encA, psA, AF.Relu, bias=ccb("b2_oa", 128))
            psB = pps.tile([128, NSUB], F32, tag="ps")
            mm(psB[0:64, :], cc("lw_oa2"), h1[2])
            mm(psB[64:112, :], cc("lw_oa2c"), h1[3][0:96, :])
            encB = penc.tile([112, NSUB], BF16, tag="encb")
            nc.vector.tensor_scalar(out=encB, in0=psB[0:112, :],
                                    scalar1=ccb("b2_oab", 112), scalar2=0.0,
                                    op0=OP.add, op1=OP.max)
            psq = pps.tile([16, NSUB], F32, tag="ps")
            mm(psq, cc("lw_en2"), hq)
            q_s = pmid.tile([16, NSUB], BF16, tag="qs")
            nc.scalar.activation(q_s, psq, AF.Relu, bias=ccb("b2_self", 16))
            psGA = pps.tile([128, NSUB], F32, tag="ps")
            mm(psGA[0:64, :], cc("lw_g2"), h1[4])
            mm(psGA[64:128, :], cc("lw_g2"), h1[5])
            encGA = penc.tile([128, NSUB], BF16, tag="enc")
            nc.scalar.activation(encGA, psGA, AF.Relu, bias=ccb("b2_g", 128))
            psGB = pps.tile([128, NSUB], F32, tag="ps")
            mm(psGB[0:64, :], cc("lw_g2"), h1[6])
            mm(psGB[64:128, :], cc("lw_g2"), h1[7])
            encGB = penc.tile([128, NSUB], BF16, tag="enc")
            nc.vector.tensor_scalar(out=encGB, in0=psGB,
                                    scalar1=ccb("b2_g", 128), scalar2=0.0,
                                    op0=OP.add, op1=OP.max)
            dump("encA", encA, 128)
            dump("encGA", encGA, 128)
            dump("q_s", q_s, 16)

            # -------------- attention scores -------------------------
            psqr = pps.tile([128, NSUB], F32, tag="ps")
            mm(psqr, cc("r16"), q_s)
            qrep = pmid.tile([128, NSUB], BF16, tag="qrep")
            nc.scalar.activation(qrep, psqr, AF.Copy, scale=0.25)
            psc = pps.tile([96, NSUB], F32, tag="ps")
            pga = pmul.tile([128, NSUB], BF16, tag="pm")
            nc.vector.tensor_mul(out=pga, in0=encGA, in1=qrep)
            mm(psc[0:32, :], cc("m8w"), pga)
            pgb = pmul.tile([128, NSUB], BF16, tag="pm")
            nc.vector.tensor_mul(out=pgb, in0=encGB, in1=qrep)
            mm(psc[32:64, :], cc("m8w"), pgb)
            poa = pmul.tile([128, NSUB], BF16, tag="pm")
            nc.vector.tensor_mul(out=poa, in0=encA, in1=qrep)
            mm(psc[64:96, :], cc("m8w"), poa, start=True, stop=False)
            pob = pmul.tile([112, NSUB], BF16, tag="pm")
            nc.vector.tensor_mul(out=pob, in0=encB, in1=qrep[0:112, :])
            mm(psc[64:96, :], cc("m8bw"), pob, start=False, stop=True)
            e_s = pmid.tile([96, NSUB], BF16, tag="es")
            nc.scalar.activation(e_s, psc, AF.Exp)
            dump("qrep", qrep, 128)
            dump("e_s", e_s, 96)

            # -------------- weighted sums ----------------------------
            att = pps.tile([64, NSUB], F32, tag="ps")
            wspec = [("e_ga", "u16w", encGA, 128, 0, True),
                     ("e_gb", "u16w", encGB, 128, 0, False),
                     ("e_oaa", "u16w", encA, 128, 32, True),
                     ("e_oab", "u16bw", encB, 112, 32, False)]
            for elh, ulh, enc_t, np_, ro, st in wspec:
                per = pps.tile([128, NSUB], F32, tag="ps")
                mm(per[0:np_, :], cc(elh), e_s)
                pp = ppn.tile([128, NSUB], BF16, tag="pp")
                nc.vector.tensor_mul(out=pp[0:np_, :], in0=enc_t,
                                     in1=per[0:np_, :])
                mm(att[ro:ro + 32, :], cc(ulh), pp[0:np_, :],
                   start=st, stop=not st)

            # ---- LN: att is already mean-centered (mask carries -1/16) ----
            d = pmid.tile([64, NSUB], F32, tag="d")
            nc.vector.tensor_scalar_add(out=d, in0=att, scalar1=0.0)
            dump("att", d, 64)
            sqd = pmid.tile([64, NSUB], BF16, tag="sqd")
            nc.scalar.activation(sqd, att, AF.Square)
            psde = pps.tile([32, NSUB], F32, tag="ps")
            mm(psde, cc("stdew"), e_s)
            deb = pnar.tile([2, NSUB], BF16, tag="deb")
            nc.scalar.activation(deb, psde[0:2, :], AF.Copy)
            de2 = pnar.tile([2, NSUB], BF16, tag="de2")
            nc.vector.tensor_mul(out=de2, in0=deb, in1=deb)
            prv = pps.tile([32, NSUB], F32, tag="ps")
            mm(prv, cc("sts2w"), sqd, start=True, stop=False)
            mm(prv, cc("id2"), de2, start=False, stop=True)
            # quake rsqrt + 1 newton step (f32, narrow)
            yi = pnar.tile([2, NSUB], I32, tag="yi")
            nc.vector.tensor_scalar(out=yi, in0=prv[0:2, :].bitcast(I32),
                                    scalar1=ccb("qshift", 2).bitcast(I32),
                                    scalar2=None, op0=OP.logical_shift_right)
            nc.vector.tensor_scalar(out=yi, in0=yi,
                                    scalar1=ccb("qxor", 2).bitcast(I32),
                                    scalar2=None, op0=OP.bitwise_xor)
            nc.vector.tensor_scalar(out=yi, in0=yi, scalar1=ccb("qmag", 2),
                                    scalar2=None, op0=OP.add)
            y0 = yi.bitcast(F32)
            t1 = pnar.tile([2, NSUB], F32, tag="t1")
            nc.vector.tensor_mul(out=t1, in0=y0, in1=y0)
            nc.vector.tensor_mul(out=t1, in0=t1, in1=prv[0:2, :])
            nc.vector.tensor_scalar(out=t1, in0=t1, scalar1=-0.5, scalar2=1.5,
                                    op0=OP.mult, op1=OP.add)
            y1 = pnar.tile([2, NSUB], BF16, tag="y1")
            nc.vector.tensor_mul(out=y1, in0=y0, in1=t1)
            pbr = pps.tile([64, NSUB], F32, tag="ps")
            mm(pbr, cc("bcrg"), y1)
            nc.vector.tensor_mul(out=d, in0=d, in1=pbr)
            mn = pmid.tile([64, NSUB], BF16, tag="mn")
            nc.scalar.activation(mn, d, AF.Relu, bias=ccb("beta64", 64))
            dump("mn", mn, 64)

            # -------------- actor ------------------------------------
            ph = pps.tile([32, NSUB], F32, tag="ps")
            mm(ph, cc("aw1s"), q_s, start=True, stop=False)
            mm(ph, cc("aw1a"), mn, start=False, stop=True)
            a1 = pmid.tile([32, NSUB], BF16, tag="a1")
            nc.scalar.activation(a1, ph, AF.Prelu, bias=ccb("ab1", 32),
                                 alpha=0.01)
            ph2 = pps.tile([32, NSUB], F32, tag="ps")
            mm(ph2, cc("aw2"), a1)
            a2 = pmid.tile([32, NSUB], BF16, tag="a2")
            nc.scalar.activation(a2, ph2, AF.Prelu, bias=ccb("ab2", 32),
                                 alpha=0.01)
            po = pps.tile([2, NSUB], F32, tag="ps")
            mm(po, cc("aw3"), a2)
            nc.scalar.activation(outw[:, s0:s0 + NSUB], po, AF.Tanh,
                                 bias=ccb("ab3", 2))

        nc.sync.dma_start(out=out[:, imac * nmac:(imac + 1) * nmac],
                          in_=outw)


def _split_excess_waits(nc):
    """Walrus in this build can encode only 1 sync wait on Activation/PE
    instruction descriptors. Move extra waits onto a NoOp just before."""
    from concourse import mybir as _mb
    nsplit = 0
    for fn in nc.m.functions:
        for bb in fn.blocks:
            insts = list(bb.instructions)
            out = []
            for ins in insts:
                si = ins.sync_info
                ow = list(si.on_wait) if (si is not None and si.on_wait) else []
                if len(ow) > 1:
                    for w0 in ow[:-1]:
                        nsplit += 1
                        nop = _mb.InstNoOp(
                            name=f"I-wsplit-{nsplit}",
                            engine=ins.engine,
                            ins=[], outs=[],
                            sync_info=_mb.SyncInfo(on_wait=[w0], on_update=[]),
                        )
                        nc.register_instruction(nop, overwrite=True)
                        out.append(nop)
                    ins.sync_info = _mb.SyncInfo(on_wait=[ow[-1]],
                                                 on_update=list(si.on_update or []))
                out.append(ins)
            if len(out) != len(insts):
                bb.instructions = out
    return nsplit


def build(nb, nf, idxb, idxf, ach):
    rc, gq, pw, nmac, nmacros = geom(ach)
    nc = bass.Bass()
    x = nc.dram_tensor("x", [96, 3 * pw], I32, kind="ExternalInput")
    xsc = nc.dram_tensor("xsc", [128, 1], F32, kind="ExternalInput")
    cwb = nc.dram_tensor("cwb", [128, nb], BF16, kind="ExternalInput")
    cwf = nc.dram_tensor("cwf", [128, nf], F32, kind="ExternalInput")
    out = nc.dram_tensor("out", [2, rc], BF16, kind="ExternalOutput")
    with tile.TileContext(nc) as tc, ExitStack() as ctx:
        _emit(nc, tc, ctx, x, xsc, cwb, cwf, out, idxb, idxf, nb, nf, ach)
    _split_excess_waits(nc)
    return nc


# ---------------------------------------------------------------- host API
class _Runtime:
    """Per-process state: bass module, jitted sharded executable, device-
    resident consts, cpu-jitted quantizer."""

    def __init__(self):
        import jax
        import jax.numpy as jnp
        from jax.sharding import Mesh, PartitionSpec, NamedSharding
        from jax.experimental.shard_map import shard_map
        from concourse import bass2jax
        self.jax = jax
        bass2jax.install_neuronx_cc_hook()

        # Bass modules (value-independent; ColPack layout fixed by shapes),
        # one per distinct chunk size in SCHED.
        zeros_w = {k: np.zeros(s, np.float32) for k, s in _WSHAPES.items()}
        cpb, cpf = build_consts(zeros_w)
        self.idxb, self.idxf = cpb.idx, cpf.idx
        self.nb, self.nf = cpb.pack().shape[1], cpf.pack().shape[1]

        devices = jax.devices()[:M]
        self.devices = devices
        self.mesh = Mesh(np.asarray(devices), ("core",))
        self.sh = NamedSharding(self.mesh, PartitionSpec("core"))
        P = PartitionSpec

        def make_run(ach):
            rc = geom(ach)[0]
            nc_ = build(self.nb, self.nf, self.idxb, self.idxf, ach)
            out_avals = [jax.core.ShapedArray((2, rc), np.dtype(BF))]
            pname = (nc_.partition_id_tensor.name
                     if nc_.partition_id_tensor else None)
            in_names = ("x", "xsc", "cwb", "cwf", "out") + (
                (pname,) if pname else ())

            def _body(x, xsc, cwb, cwf, z):
                operands = [x, xsc, cwb, cwf, z]
                if pname is not None:
                    operands.append(bass2jax.partition_id_tensor())
                outs = bass2jax._bass_exec_p.bind(
                    *operands,
                    out_avals=tuple(out_avals),
                    in_names=in_names,
                    out_names=("out",),
                    lowering_input_output_aliases=(),
                    sim_require_finite=True,
                    sim_require_nnan=True,
                    nc=nc_)
                return tuple(outs)

            run = jax.jit(
                shard_map(_body, mesh=self.mesh,
                          in_specs=(P("core"),) * 5, out_specs=(P("core"),),
                          check_rep=False),
                donate_argnums=(4,), keep_unused=True)
            mkzeros = jax.jit(
                lambda: jnp.zeros((M * 2, rc), jnp.bfloat16),
                out_shardings=self.sh)
            return run, mkzeros

        self.runs = {ach: make_run(ach) for ach in sorted(set(SCHED))}
        self.wkey = None
        self.cwb_d = None
        self.cwf_d = None

        # numpy quantize scratch per chunk size; per-(chunk,core) block
        # stays L2-resident
        self._scr = {}
        for ach in set(SCHED):
            rc, gq, pw, _, _ = geom(ach)
            self._scr[ach] = (
                np.empty((ach, Bs, OBS), np.float32),    # den
                np.empty((ach, Bs, OBS), np.float32),    # num
                np.empty((ach * Bs, OBS), np.uint8),     # tmp8
                np.empty((OBS, rc), np.uint8),           # ct
                np.empty((OBS, gq), np.uint8),           # t1
                np.empty((OBS, gq), np.uint8),           # t2
            )

    def prep_np(self, s, a0, ach):
        """Encode agents [a0, a0+ach) of s [A,B,OBS] f32 -> (xp
        [M*96, 3*pw] int32 bit-planes of 6-bit companded codes,
        xsc [M*128,1] f32 = per-core scale).

        Scale is a sampled absmax (stride 64); the explicit clip bounds
        |y| <= 31 so the 1.5*2^23 round trick stays exact. Clipping the
        few unsampled outliers slightly tightens the quant steps and
        measurably improves accuracy vs the exact absmax."""
        rc, gq, pw, _, _ = geom(ach)
        den, num, tmp8, ct, t1, t2 = self._scr[ach]
        xp = np.empty((M * OBS, 3 * pw), np.int32)
        xpu = xp.view(np.uint8).reshape(M * OBS, 3 * gq)
        xsc = np.empty((M * 128, 1), np.float32)
        iv = num.view(np.int32)
        t8 = tmp8.reshape(ach, Bs, OBS)
        for m in range(M):
            src = s[a0:a0 + ach, m * Bs:(m + 1) * Bs, :]
            sub = src.reshape(-1)[::64]
            amax = max(float(sub.max()), -float(sub.min()), 1e-20)
            # y = c1*x/(d + |x|) with d = amax/AV, c1 = 31*(1+AV)/AV
            np.abs(src, out=den)
            np.add(den, np.float32(amax / AV), out=den)
            np.multiply(src, np.float32(31.0 * (1.0 + AV) / AV), out=num)
            np.divide(num, den, out=num)
            np.clip(num, -31.0, 31.0, out=num)
            np.add(num, np.float32(12582912.0), out=num)
            np.subtract(iv, np.int32(0x4B400000 - 32), out=iv)
            np.copyto(t8, iv, casting='unsafe')
            ct[...] = tmp8.T
            A0 = ct[:, 0:gq]
            A1 = ct[:, gq:2 * gq]
            A2 = ct[:, 2 * gq:3 * gq]
            A3 = ct[:, 3 * gq:4 * gq]
            W = xpu[m * OBS:(m + 1) * OBS]
            np.left_shift(A1, 6, out=t1)
            np.bitwise_or(A0, t1, out=W[:, 0:gq])
            np.right_shift(A1, 2, out=t1)
            np.left_shift(A2, 4, out=t2)
            np.bitwise_or(t1, t2, out=W[:, gq:2 * gq])
            np.right_shift(A2, 4, out=t1)
            np.left_shift(A3, 2, out=t2)
            np.bitwise_or(t1, t2, out=W[:, 2 * gq:3 * gq])
            xsc[m * 128:(m + 1) * 128] = np.float32(amax)
        return xp, xsc

    def prep_put_early(self, s, a0, ach):
        """Like prep_np, but device_puts each core's block to its device
        as soon as it is packed, so the first bytes hit the axon link
        ~3 ms in instead of after the whole chunk is encoded. Returns
        committed sharded jax arrays assembled zero-copy from the
        per-device pieces."""
        jax = self.jax
        rc, gq, pw, _, _ = geom(ach)
        xp, xsc = None, None
        den, num, tmp8, ct, t1, t2 = self._scr[ach]
        iv = num.view(np.int32)
        t8 = tmp8.reshape(ach, Bs, OBS)
        blk = np.empty((OBS, 3 * pw), np.int32)
        blku = blk.view(np.uint8).reshape(OBS, 3 * gq)
        scb = np.empty((128, 1), np.float32)
        ds, dscs = [], []
        for m in range(M):
            src = s[a0:a0 + ach, m * Bs:(m + 1) * Bs, :]
            sub = src.reshape(-1)[::64]
            amax = max(float(sub.max()), -float(sub.min()), 1e-20)
            np.abs(src, out=den)
            np.add(den, np.float32(amax / AV), out=den)
            np.multiply(src, np.float32(31.0 * (1.0 + AV) / AV), out=num)
            np.divide(num, den, out=num)
            np.clip(num, -31.0, 31.0, out=num)
            np.add(num, np.float32(12582912.0), out=num)
            np.subtract(iv, np.int32(0x4B400000 - 32), out=iv)
            np.copyto(t8, iv, casting='unsafe')
            ct[...] = tmp8.T
            A0 = ct[:, 0:gq]
            A1 = ct[:, gq:2 * gq]
            A2 = ct[:, 2 * gq:3 * gq]
            A3 = ct[:, 3 * gq:4 * gq]
            np.left_shift(A1, 6, out=t1)
            np.bitwise_or(A0, t1, out=blku[:, 0:gq])
            np.right_shift(A1, 2, out=t1)
            np.left_shift(A2, 4, out=t2)
            np.bitwise_or(t1, t2, out=blku[:, gq:2 * gq])
            np.right_shift(A2, 4, out=t1)
            np.left_shift(A3, 2, out=t2)
            np.bitwise_or(t1, t2, out=blku[:, 2 * gq:3 * gq])
            scb[:] = np.float32(amax)
            ds.append(jax.device_put(blk, self.devices[m]))
            dscs.append(jax.device_put(scb, self.devices[m]))
        xp = jax.make_array_from_single_device_arrays(
            (M * OBS, 3 * pw), self.sh, ds)
        xsc = jax.make_array_from_single_device_arrays(
            (M * 128, 1), self.sh, dscs)
        return xp, xsc

    def set_weights(self, w):
        import hashlib
        h = hashlib.blake2b(digest_size=16)
        for k in sorted(_WSHAPES):
            h.update(np.ascontiguousarray(w[k], np.float32).tobytes())
        key = h.digest()
        if key == self.wkey:
            return
        cpb, cpf = build_consts(w)
        assert cpb.idx == self.idxb and cpf.idx == self.idxf
        cwb = np.ascontiguousarray(cpb.pack().astype(BF))
        cwf = np.ascontiguousarray(cpf.pack())
        self.cwb_d = self.jax.device_put(
            np.concatenate([cwb] * M, axis=0), self.sh)
        self.cwf_d = self.jax.device_put(
            np.concatenate([cwf] * M, axis=0), self.sh)
        self.jax.block_until_ready((self.cwb_d, self.cwf_d))
        self.wkey = key


_WSHAPES = {
    "en_w1": (4, NU), "en_b1": (NU,), "en_w2": (NU, NT), "en_b2": (NT,),
    "oa_w1": (4, NU), "oa_b1": (NU,), "oa_w2": (NU, NT), "oa_b2": (NT,),
    "oa_ln_g": (NT,), "oa_ln_b": (NT,),
    "g_w1": (2, NU), "g_b1": (NU,), "g_w2": (NU, NT), "g_b2": (NT,),
    "g_ln_g": (NT,), "g_ln_b": (NT,),
    "a_w1": (NT * 3, NU), "a_b1": (NU,), "a_w2": (NU, NU), "a_b2": (NU,),
    "a_w3": (NU, 2), "a_b3": (2,),
}

_rt = None


def _runtime():
    global _rt
    if _rt is None:
        _rt = _Runtime()
    return _rt


def kernel(**inputs):
    rt = _runtime()
    w = {k: np.asarray(inputs[k], np.float32) for k in _WSHAPES}
    rt.set_weights(w)
    s = np.asarray(inputs["s_input"], np.float32)

    # Interleaved pipeline: encode chunk c (numpy, mostly overlapping the
    # axon link streaming earlier chunks), enqueue its device call
    # immediately so the network never starves. The jit enqueue copies xp
    # into PJRT staging synchronously, so the numpy scratch buffers are
    # free to reuse right after run() returns.
    outs = []
    a0 = 0
    for ci, ach in enumerate(SCHED):
        if ci == 0:
            # per-core streaming puts: first bytes hit the wire ~3 ms in
            xp, xsc = rt.prep_put_early(s, a0, ach)
        else:
            xp, xsc = rt.prep_np(s, a0, ach)
        run, mkzeros = rt.runs[ach]
        # zero-fill dispatch is device-side (no wire bytes); issued here,
        # after the puts are already streaming, it costs no link idle time
        (o,) = run(xp, xsc, rt.cwb_d, rt.cwf_d, mkzeros())
        o.copy_to_host_async()
        outs.append((a0, ach, o))
        a0 += ach

    res = np.empty((A, B, 2), np.float32)
    for a0, ach, o in outs:
        v = np.asarray(o).astype(np.float32)          # [M*2, rc]
        v = v.reshape(M, 2, ach, Bs)                  # [m, j, a_l, b]
        res[a0:a0 + ach] = v.transpose(2, 0, 3, 1).reshape(ach, B, 2)
    return res

